# revision 4
# baseline (speedup 1.0000x reference)
"""DSAttention Trainium2 kernel.

Reference computation (per batch b, head h):
    S[q,s]  = (Q[q]·K[s]) * tau[b] + delta[b,s]
    S      += causal mask (s > q -> -inf)
    A       = softmax(S / sqrt(E), axis=s)
    O[q,:]  = sum_s A[q,s] * V[s,:]

Shapes: B=2, L=2048, H=16, E=64 -> 32 (b,h) pairs, 4 per NeuronCore x 8 cores.

Device strategy (per core, per head):
  - Q^T, K^T resident in SBUF as [e=64, L] (host pre-transposed).
  - V with a ones-column appended: [128, 16 chunks, 65]; the AV matmul then
    produces both O^T (rows 0..63) and the softmax denominator (row 64) in
    one PSUM accumulation.
  - Loop over s-chunks n (128 keys each): S^T[s,q] = K_n^T.T @ Q^T computed
    only for q >= 128n (causal skip), as <=512-wide matmul pieces into a
    [128,1024] PSUM tile (2 halves of the q range, double buffered).
  - One Exp activation per (chunk, half): exp(dot * (tau/8) + delta/8) with
    tau as a per-partition scale AP and delta/8 as a per-partition bias AP.
    No max-subtraction: |dot*tau/8 + delta/8| <= ~8, exp is safe in fp32.
  - Diagonal 128x128 block masked by multiplying with an upper-triangular
    0/1 matrix after the exp.
  - AV: O^T[65, q] += V1_n.T @ A^T accumulated over chunks in a [65, 2048]
    PSUM tile; when a 512-column q-tile is complete, divide rows 0..63 by
    row 64 (reciprocal + partition-broadcast DMA + scalar_tensor_tensor)
    and DMA out.  Output is O^T [64, L] per head; host transposes back.
  - Matmuls run in float32r (full-rate fp32 PE mode).
"""

import sys

sys.path.insert(0, "/opt/trn_rl_repo")

import ml_dtypes
import numpy as np

import concourse.bass as bass
import concourse.tile as tile
from concourse import bacc, mybir
from concourse.masks import make_upper_triangular

B, L, H, E = 2, 2048, 16, 64
NCORES = 8
HPC = (B * H) // NCORES  # heads per core = 4
NCH = L // 128  # s-chunks per head = 16
SCALE = 1.0 / 8.0  # 1/sqrt(E)
F32 = mybir.dt.float32
F32R = mybir.dt.float32r
BF16 = mybir.dt.bfloat16
EXP = mybir.ActivationFunctionType.Exp
MULT = mybir.AluOpType.mult


def _pieces(n):
    """512-aligned matmul pieces covering q in [128n, L)."""
    q0 = 128 * n
    out = []
    c = q0
    w = 512 - 128 * (n % 4)
    out.append((c, w))
    c += w
    while c < L:
        out.append((c, 512))
        c += 512
    return out


def _body(tc, qT, kT, v1, dlt, tau_in, out):
    nc = tc.nc
    from contextlib import ExitStack

    with ExitStack() as ctx:
        const = ctx.enter_context(tc.tile_pool(name="const", bufs=1))
        qk_pool = ctx.enter_context(tc.tile_pool(name="qk", bufs=2))
        v_pool = ctx.enter_context(tc.tile_pool(name="v", bufs=2))
        hd_pool = ctx.enter_context(tc.tile_pool(name="hd", bufs=2))
        a_pool = ctx.enter_context(tc.tile_pool(name="a", bufs=3))
        o_pool = ctx.enter_context(tc.tile_pool(name="o", bufs=2))
        r_pool = ctx.enter_context(tc.tile_pool(name="r", bufs=2))
        ps_pool = ctx.enter_context(tc.tile_pool(name="psS", bufs=2, space="PSUM"))
        po_pool = ctx.enter_context(tc.tile_pool(name="psO", bufs=1, space="PSUM"))

        trimask = const.tile([128, 128], BF16, name="trimask")
        make_upper_triangular(nc, trimask[:], val=1.0, diag=True)

        for i in range(HPC):
            qt = qk_pool.tile([64, L], F32R, tag="qt", name=f"qt{i}")
            nc.sync.dma_start(qt[:], qT[i])
            kt = qk_pool.tile([64, L], F32R, tag="kt", name=f"kt{i}")
            nc.sync.dma_start(kt[:], kT[i])
            vt = v_pool.tile([128, NCH * 65], BF16, tag="vt", name=f"vt{i}")
            nc.sync.dma_start(vt[:], v1[i])
            dl = hd_pool.tile([128, NCH], F32, tag="dl", name=f"dl{i}")
            nc.sync.dma_start(dl[:], dlt[i])
            dls = hd_pool.tile([128, NCH], F32, tag="dls", name=f"dls{i}")
            nc.vector.tensor_scalar_mul(dls[:], dl[:], SCALE)
            tt = hd_pool.tile([128, 1], F32, tag="tt", name=f"tt{i}")
            nc.sync.dma_start(tt[:], tau_in[i : i + 1, 0:1].to_broadcast([128, 1]))
            tts = hd_pool.tile([128, 1], F32, tag="tts", name=f"tts{i}")
            nc.vector.tensor_scalar_mul(tts[:], tt[:], SCALE)

            oT = po_pool.tile([65, L], F32, tag="oT", name=f"oT{i}")  # 4 PSUM banks
            o_sb = o_pool.tile([64, L], F32, tag="osb", name=f"osb{i}")


            for n in range(NCH):
                q0 = 128 * n
                pieces = _pieces(n)
                halves = [h for h in range(2) if 1024 * (h + 1) > q0]
                ps = {}
                for h in halves:
                    ps[h] = ps_pool.tile(
                        [128, 1024], F32, tag="ps", name=f"ps{i}_{n}_{h}"
                    )
                for c0, w in pieces:
                    h = c0 // 1024
                    nc.tensor.matmul(
                        ps[h][:, c0 - 1024 * h : c0 - 1024 * h + w],
                        lhsT=kt[:, q0 : q0 + 128],
                        rhs=qt[:, c0 : c0 + w],
                        start=True,
                        stop=True,
                    )
                a_sb = a_pool.tile([128, L], BF16, tag="a", name=f"a{i}_{n}")
                for h in halves:
                    lo = max(q0, 1024 * h)
                    hi = 1024 * (h + 1)
                    nc.scalar.activation(
                        a_sb[:, lo:hi],
                        ps[h][:, lo - 1024 * h : hi - 1024 * h],
                        EXP,
                        bias=dls[:, n : n + 1],
                        scale=tts[:, 0:1],
                    )
                nc.vector.tensor_mul(
                    a_sb[:, q0 : q0 + 128], a_sb[:, q0 : q0 + 128], trimask[:]
                )
                for c0, w in pieces:
                    j = c0 // 512
                    nc.tensor.matmul(
                        oT[:, c0 : c0 + w],
                        lhsT=vt[:, n * 65 : n * 65 + 65],
                        rhs=a_sb[:, c0 : c0 + w],
                        start=(n == 0),
                        stop=(n == 4 * j + 3),
                    )
                if n % 4 == 3:
                    # q-tile j is fully accumulated; normalize and store.
                    j = (n - 3) // 4
                    sl = slice(512 * j, 512 * (j + 1))
                    rrow = r_pool.tile([1, 512], F32, tag="rr", name=f"rr{i}_{j}")
                    nc.vector.reciprocal(rrow[:], oT[64:65, sl])
                    rbc = r_pool.tile([64, 512], F32, tag="rbc", name=f"rbc{i}_{j}")
                    nc.gpsimd.partition_broadcast(rbc[:], rrow[:], channels=64)
                    nc.vector.scalar_tensor_tensor(
                        out=o_sb[:, sl],
                        in0=oT[0:64, sl],
                        scalar=1.0,
                        in1=rbc[:],
                        op0=MULT,
                        op1=MULT,
                    )
                    nc.sync.dma_start(out[i][:, sl], o_sb[:, sl])


_CACHED = None


def _build():
    global _CACHED
    if _CACHED is not None:
        return _CACHED
    nc = bacc.Bacc("TRN2", target_bir_lowering=False, debug=False)
    qT = nc.dram_tensor("qT", [HPC, 64, L], F32R, kind="ExternalInput").ap()
    kT = nc.dram_tensor("kT", [HPC, 64, L], F32R, kind="ExternalInput").ap()
    v1 = nc.dram_tensor("v1", [HPC, 128, NCH * 65], BF16, kind="ExternalInput").ap()
    dlt = nc.dram_tensor("dlt", [HPC, 128, NCH], F32, kind="ExternalInput").ap()
    tau_in = nc.dram_tensor("tau_in", [HPC, 1], F32, kind="ExternalInput").ap()
    out = nc.dram_tensor("out", [HPC, 64, L], F32, kind="ExternalOutput").ap()
    with tile.TileContext(nc) as tc:
        _body(tc, qT, kT, v1, dlt, tau_in, out)
    nc.compile()
    _CACHED = nc
    return nc


def _prep_in_maps(queries, keys, values, tau, delta):
    """Shard + relayout the full inputs into 8 per-core input dicts."""
    queries = np.asarray(queries, dtype=np.float32)
    keys = np.asarray(keys, dtype=np.float32)
    values = np.asarray(values, dtype=np.float32)
    tau = np.asarray(tau, dtype=np.float32)
    delta = np.asarray(delta, dtype=np.float32)

    in_maps = []
    for core in range(NCORES):
        qTs = np.empty((HPC, 64, L), np.float32)
        kTs = np.empty((HPC, 64, L), np.float32)
        v1s = np.empty((HPC, 128, NCH * 65), ml_dtypes.bfloat16)
        dls = np.empty((HPC, 128, NCH), np.float32)
        tas = np.empty((HPC, 1), np.float32)
        for slot in range(HPC):
            g = core * HPC + slot
            b, h = divmod(g, H)
            qTs[slot] = queries[b, :, h, :].T
            kTs[slot] = keys[b, :, h, :].T
            v = values[b, :, h, :].reshape(NCH, 128, E).transpose(1, 0, 2)
            vv = np.concatenate([v, np.ones((128, NCH, 1), np.float32)], axis=2)
            v1s[slot] = vv.reshape(128, NCH * 65).astype(ml_dtypes.bfloat16)
            dls[slot] = delta[b].reshape(NCH, 128).T
            tas[slot, 0] = tau[b, 0]
        in_maps.append(
            {"qT": qTs, "kT": kTs, "v1": v1s, "dlt": dls, "tau_in": tas}
        )
    return in_maps


def _assemble(results):
    O = np.empty((B, L, H, E), np.float32)
    for core in range(NCORES):
        o = results[core]["out"]  # [HPC, 64, L]
        for slot in range(HPC):
            g = core * HPC + slot
            b, h = divmod(g, H)
            O[b, :, h, :] = o[slot].T
    return O


def run(inputs, trace=False, **kwargs):
    from concourse import bass_utils

    nc = _build()
    in_maps = _prep_in_maps(**inputs)
    res = bass_utils.run_bass_kernel_spmd(
        nc, in_maps, core_ids=list(range(NCORES)), trace=trace, **kwargs
    )
    return _assemble(res.results), res


def kernel(**inputs):
    return run(inputs, trace=False)[0]


# revision 5
# speedup vs baseline: 1.1775x; 1.1775x over previous
"""DSAttention Trainium2 kernel.

Reference computation (per batch b, head h):
    S[q,s]  = (Q[q]·K[s]) * tau[b] + delta[b,s]
    S      += causal mask (s > q -> -inf)
    A       = softmax(S / sqrt(E), axis=s)
    O[q,:]  = sum_s A[q,s] * V[s,:]

Shapes: B=2, L=2048, H=16, E=64 -> 32 (b,h) pairs, 4 per NeuronCore x 8 cores.

Device strategy (per core, per head):
  - Q^T, K^T resident in SBUF as [e=64, L] (host pre-transposed).
  - V with a ones-column appended: [128, 16 chunks, 65]; the AV matmul then
    produces both O^T (rows 0..63) and the softmax denominator (row 64) in
    one PSUM accumulation.
  - Loop over s-chunks n (128 keys each): S^T[s,q] = K_n^T.T @ Q^T computed
    only for q >= 128n (causal skip), as <=512-wide matmul pieces into a
    [128,1024] PSUM tile (2 halves of the q range, double buffered).
  - One Exp activation per (chunk, half): exp(dot * (tau/8) + delta/8) with
    tau as a per-partition scale AP and delta/8 as a per-partition bias AP.
    No max-subtraction: |dot*tau/8 + delta/8| <= ~8, exp is safe in fp32.
  - Diagonal 128x128 block masked by multiplying with an upper-triangular
    0/1 matrix after the exp.
  - AV: O^T[65, q] += V1_n.T @ A^T accumulated over chunks in a [65, 2048]
    PSUM tile; when a 512-column q-tile is complete, divide rows 0..63 by
    row 64 (reciprocal + partition-broadcast DMA + scalar_tensor_tensor)
    and DMA out.  Output is O^T [64, L] per head; host transposes back.
  - Matmuls run in float32r (full-rate fp32 PE mode).
"""

import sys

sys.path.insert(0, "/opt/trn_rl_repo")

import ml_dtypes
import numpy as np

import concourse.bass as bass
import concourse.tile as tile
from concourse import bacc, mybir
from concourse.masks import make_identity, make_upper_triangular

B, L, H, E = 2, 2048, 16, 64
NCORES = 8
HPC = (B * H) // NCORES  # heads per core = 4
NCH = L // 128  # s-chunks per head = 16
SCALE = 1.0 / 8.0  # 1/sqrt(E)
F32 = mybir.dt.float32
F32R = mybir.dt.float32r
BF16 = mybir.dt.bfloat16
EXP = mybir.ActivationFunctionType.Exp
MULT = mybir.AluOpType.mult


def _pieces(n):
    """512-aligned matmul pieces covering q in [128n, L)."""
    q0 = 128 * n
    out = []
    c = q0
    w = 512 - 128 * (n % 4)
    out.append((c, w))
    c += w
    while c < L:
        out.append((c, 512))
        c += 512
    return out


def _body(tc, qT, kT, v1, dlt, tau_in, out):
    nc = tc.nc
    from contextlib import ExitStack

    with ExitStack() as ctx:
        const = ctx.enter_context(tc.tile_pool(name="const", bufs=1))
        qk_pool = ctx.enter_context(tc.tile_pool(name="qk", bufs=2))
        v_pool = ctx.enter_context(tc.tile_pool(name="v", bufs=2))
        hd_pool = ctx.enter_context(tc.tile_pool(name="hd", bufs=2))
        a_pool = ctx.enter_context(tc.tile_pool(name="a", bufs=3))
        o_pool = ctx.enter_context(tc.tile_pool(name="o", bufs=2))
        r_pool = ctx.enter_context(tc.tile_pool(name="r", bufs=2))
        ps_pool = ctx.enter_context(tc.tile_pool(name="psS", bufs=2, space="PSUM"))
        po_pool = ctx.enter_context(tc.tile_pool(name="psO", bufs=1, space="PSUM"))

        trimask = const.tile([128, 128], BF16, name="trimask")
        make_upper_triangular(nc, trimask[:], val=1.0, diag=True)
        ident65 = const.tile([65, 65], F32, name="ident65")
        make_identity(nc, ident65[:])

        for i in range(HPC):
            qt = qk_pool.tile([64, L], F32R, tag="qt", name=f"qt{i}")
            nc.sync.dma_start(qt[:], qT[i])
            kt = qk_pool.tile([64, L], F32R, tag="kt", name=f"kt{i}")
            nc.sync.dma_start(kt[:], kT[i])
            vt = v_pool.tile([128, NCH * 65], BF16, tag="vt", name=f"vt{i}")
            nc.sync.dma_start(vt[:], v1[i])
            dl = hd_pool.tile([128, NCH], F32, tag="dl", name=f"dl{i}")
            nc.sync.dma_start(dl[:], dlt[i])
            dls = hd_pool.tile([128, NCH], F32, tag="dls", name=f"dls{i}")
            nc.vector.tensor_scalar_mul(dls[:], dl[:], SCALE)
            tt = hd_pool.tile([128, 1], F32, tag="tt", name=f"tt{i}")
            nc.sync.dma_start(tt[:], tau_in[i : i + 1, 0:1].to_broadcast([128, 1]))
            tts = hd_pool.tile([128, 1], F32, tag="tts", name=f"tts{i}")
            nc.vector.tensor_scalar_mul(tts[:], tt[:], SCALE)

            oT = po_pool.tile([65, L], F32, tag="oT", name=f"oT{i}")  # 4 PSUM banks
            o_sb = o_pool.tile([65, L], F32, tag="osb", name=f"osb{i}")


            for n in range(NCH):
                q0 = 128 * n
                pieces = _pieces(n)
                halves = [h for h in range(2) if 1024 * (h + 1) > q0]
                ps = {}
                for h in halves:
                    ps[h] = ps_pool.tile(
                        [128, 1024], F32, tag="ps", name=f"ps{i}_{n}_{h}"
                    )
                for c0, w in pieces:
                    h = c0 // 1024
                    nc.tensor.matmul(
                        ps[h][:, c0 - 1024 * h : c0 - 1024 * h + w],
                        lhsT=kt[:, q0 : q0 + 128],
                        rhs=qt[:, c0 : c0 + w],
                        start=True,
                        stop=True,
                    )
                a_sb = a_pool.tile([128, L], BF16, tag="a", name=f"a{i}_{n}")
                for h in halves:
                    lo = max(q0, 1024 * h)
                    hi = 1024 * (h + 1)
                    nc.scalar.activation(
                        a_sb[:, lo:hi],
                        ps[h][:, lo - 1024 * h : hi - 1024 * h],
                        EXP,
                        bias=dls[:, n : n + 1],
                        scale=tts[:, 0:1],
                    )
                nc.vector.tensor_mul(
                    a_sb[:, q0 : q0 + 128], a_sb[:, q0 : q0 + 128], trimask[:]
                )
                for c0, w in pieces:
                    j = c0 // 512
                    nc.tensor.matmul(
                        oT[:, c0 : c0 + w],
                        lhsT=vt[:, n * 65 : n * 65 + 65],
                        rhs=a_sb[:, c0 : c0 + w],
                        start=(n == 0),
                        stop=(n == 4 * j + 3),
                    )
            # Head finalization: copy O^T' to SBUF, PE-transpose per 128-q
            # chunk so the denominator (row 64) becomes a per-partition
            # scalar, then reciprocal + scaled copy + store in [q, d] layout.
            nc.vector.tensor_copy(o_sb[:], oT[:, :])
            psT = po_pool.tile([128, L], F32, tag="oT", name=f"psT{i}")
            for t in range(NCH):
                nc.tensor.transpose(
                    psT[:, 128 * t : 128 * t + 65],
                    o_sb[:, 128 * t : 128 * t + 128],
                    ident65[:],
                )
            psT3 = psT.rearrange("p (t c) -> p t c", c=128)
            recipv = r_pool.tile([128, NCH], F32, tag="rr", name=f"rr{i}")
            nc.vector.reciprocal(recipv[:], psT3[:, :, 64])
            o2 = o_pool.tile([128, NCH, 64], F32, tag="o2", name=f"o2_{i}")
            nc.vector.scalar_tensor_tensor(
                out=o2[:],
                in0=psT3[:, :, 0:64],
                scalar=1.0,
                in1=recipv[:, :, None].to_broadcast([128, NCH, 64]),
                op0=MULT,
                op1=MULT,
            )
            nc.sync.dma_start(
                out[i].rearrange("(t p) d -> p t d", p=128), o2[:]
            )


_CACHED = None


def _build():
    global _CACHED
    if _CACHED is not None:
        return _CACHED
    nc = bacc.Bacc("TRN2", target_bir_lowering=False, debug=False)
    qT = nc.dram_tensor("qT", [HPC, 64, L], F32R, kind="ExternalInput").ap()
    kT = nc.dram_tensor("kT", [HPC, 64, L], F32R, kind="ExternalInput").ap()
    v1 = nc.dram_tensor("v1", [HPC, 128, NCH * 65], BF16, kind="ExternalInput").ap()
    dlt = nc.dram_tensor("dlt", [HPC, 128, NCH], F32, kind="ExternalInput").ap()
    tau_in = nc.dram_tensor("tau_in", [HPC, 1], F32, kind="ExternalInput").ap()
    out = nc.dram_tensor("out", [HPC, L, E], F32, kind="ExternalOutput").ap()
    with tile.TileContext(nc) as tc:
        _body(tc, qT, kT, v1, dlt, tau_in, out)
    nc.compile()
    _CACHED = nc
    return nc


def _prep_in_maps(queries, keys, values, tau, delta):
    """Shard + relayout the full inputs into 8 per-core input dicts."""
    queries = np.asarray(queries, dtype=np.float32)
    keys = np.asarray(keys, dtype=np.float32)
    values = np.asarray(values, dtype=np.float32)
    tau = np.asarray(tau, dtype=np.float32)
    delta = np.asarray(delta, dtype=np.float32)

    in_maps = []
    for core in range(NCORES):
        qTs = np.empty((HPC, 64, L), np.float32)
        kTs = np.empty((HPC, 64, L), np.float32)
        v1s = np.empty((HPC, 128, NCH * 65), ml_dtypes.bfloat16)
        dls = np.empty((HPC, 128, NCH), np.float32)
        tas = np.empty((HPC, 1), np.float32)
        for slot in range(HPC):
            g = core * HPC + slot
            b, h = divmod(g, H)
            qTs[slot] = queries[b, :, h, :].T
            kTs[slot] = keys[b, :, h, :].T
            v = values[b, :, h, :].reshape(NCH, 128, E).transpose(1, 0, 2)
            vv = np.concatenate([v, np.ones((128, NCH, 1), np.float32)], axis=2)
            v1s[slot] = vv.reshape(128, NCH * 65).astype(ml_dtypes.bfloat16)
            dls[slot] = delta[b].reshape(NCH, 128).T
            tas[slot, 0] = tau[b, 0]
        in_maps.append(
            {"qT": qTs, "kT": kTs, "v1": v1s, "dlt": dls, "tau_in": tas}
        )
    return in_maps


def _assemble(results):
    O = np.empty((B, L, H, E), np.float32)
    for core in range(NCORES):
        o = results[core]["out"]  # [HPC, L, E]
        for slot in range(HPC):
            g = core * HPC + slot
            b, h = divmod(g, H)
            O[b, :, h, :] = o[slot]
    return O


def run(inputs, trace=False, **kwargs):
    from concourse import bass_utils

    nc = _build()
    in_maps = _prep_in_maps(**inputs)
    res = bass_utils.run_bass_kernel_spmd(
        nc, in_maps, core_ids=list(range(NCORES)), trace=trace, **kwargs
    )
    return _assemble(res.results), res


def kernel(**inputs):
    return run(inputs, trace=False)[0]


# revision 6
# speedup vs baseline: 1.2995x; 1.1036x over previous
"""DSAttention Trainium2 kernel.

Reference computation (per batch b, head h):
    S[q,s]  = (Q[q]·K[s]) * tau[b] + delta[b,s]
    S      += causal mask (s > q -> -inf)
    A       = softmax(S / sqrt(E), axis=s)
    O[q,:]  = sum_s A[q,s] * V[s,:]

Shapes: B=2, L=2048, H=16, E=64 -> 32 (b,h) pairs, 4 per NeuronCore x 8 cores.

Device strategy (per core, per head):
  - Q^T, K^T resident in SBUF as [e=64, L] (host pre-transposed).
  - V with a ones-column appended: [128, 16 chunks, 65]; the AV matmul then
    produces both O^T (rows 0..63) and the softmax denominator (row 64) in
    one PSUM accumulation.
  - Loop over s-chunks n (128 keys each): S^T[s,q] = K_n^T.T @ Q^T computed
    only for q >= 128n (causal skip), as <=512-wide matmul pieces into a
    [128,1024] PSUM tile (2 halves of the q range, double buffered).
  - One Exp activation per (chunk, half): exp(dot * (tau/8) + delta/8) with
    tau as a per-partition scale AP and delta/8 as a per-partition bias AP.
    No max-subtraction: |dot*tau/8 + delta/8| <= ~8, exp is safe in fp32.
  - Diagonal 128x128 block masked by multiplying with an upper-triangular
    0/1 matrix after the exp.
  - AV: O^T[65, q] += V1_n.T @ A^T accumulated over chunks in a [65, 2048]
    PSUM tile; when a 512-column q-tile is complete, divide rows 0..63 by
    row 64 (reciprocal + partition-broadcast DMA + scalar_tensor_tensor)
    and DMA out.  Output is O^T [64, L] per head; host transposes back.
  - Matmuls run in float32r (full-rate fp32 PE mode).
"""

import sys

sys.path.insert(0, "/opt/trn_rl_repo")

import ml_dtypes
import numpy as np

import concourse.bass as bass
import concourse.tile as tile
from concourse import bacc, mybir
from concourse.masks import make_identity, make_upper_triangular

B, L, H, E = 2, 2048, 16, 64
NCORES = 8
HPC = (B * H) // NCORES  # heads per core = 4
NCH = L // 128  # s-chunks per head = 16
SCALE = 1.0 / 8.0  # 1/sqrt(E)
F32 = mybir.dt.float32
F32R = mybir.dt.float32r
BF16 = mybir.dt.bfloat16
EXP = mybir.ActivationFunctionType.Exp
MULT = mybir.AluOpType.mult


def _pieces(n):
    """512-aligned matmul pieces covering q in [128n, L)."""
    q0 = 128 * n
    out = []
    c = q0
    w = 512 - 128 * (n % 4)
    out.append((c, w))
    c += w
    while c < L:
        out.append((c, 512))
        c += 512
    return out


def _body(tc, qT, kT, v1, dlt, tau_in, out):
    nc = tc.nc
    from contextlib import ExitStack

    with ExitStack() as ctx:
        const = ctx.enter_context(tc.tile_pool(name="const", bufs=1))
        qk_pool = ctx.enter_context(tc.tile_pool(name="qk", bufs=2))
        v_pool = ctx.enter_context(tc.tile_pool(name="v", bufs=2))
        hd_pool = ctx.enter_context(tc.tile_pool(name="hd", bufs=2))
        a_pool = ctx.enter_context(tc.tile_pool(name="a", bufs=3))
        o_pool = ctx.enter_context(tc.tile_pool(name="o", bufs=2))
        r_pool = ctx.enter_context(tc.tile_pool(name="r", bufs=2))
        ps_pool = ctx.enter_context(tc.tile_pool(name="psS", bufs=2, space="PSUM"))
        po_pool = ctx.enter_context(tc.tile_pool(name="psO", bufs=1, space="PSUM"))

        trimask = const.tile([128, 128], BF16, name="trimask")
        make_upper_triangular(nc, trimask[:], val=1.0, diag=True)
        ident65 = const.tile([65, 65], F32, name="ident65")
        make_identity(nc, ident65[:])

        for i in range(HPC):
            qt = qk_pool.tile([64, L], BF16, tag="qt", name=f"qt{i}")
            nc.sync.dma_start(qt[:], qT[i])
            kt = qk_pool.tile([64, L], BF16, tag="kt", name=f"kt{i}")
            nc.sync.dma_start(kt[:], kT[i])
            vt = v_pool.tile([128, NCH * 65], BF16, tag="vt", name=f"vt{i}")
            nc.sync.dma_start(vt[:], v1[i])
            dl = hd_pool.tile([128, NCH], F32, tag="dl", name=f"dl{i}")
            nc.sync.dma_start(dl[:], dlt[i])
            dls = hd_pool.tile([128, NCH], F32, tag="dls", name=f"dls{i}")
            nc.vector.tensor_scalar_mul(dls[:], dl[:], SCALE)
            tt = hd_pool.tile([128, 1], F32, tag="tt", name=f"tt{i}")
            nc.sync.dma_start(tt[:], tau_in[i : i + 1, 0:1].to_broadcast([128, 1]))
            tts = hd_pool.tile([128, 1], F32, tag="tts", name=f"tts{i}")
            nc.vector.tensor_scalar_mul(tts[:], tt[:], SCALE)

            oT = po_pool.tile([65, L], F32, tag="oT", name=f"oT{i}")  # 4 PSUM banks
            o_sb = o_pool.tile([65, L], F32, tag="osb", name=f"osb{i}")


            for n in range(NCH):
                q0 = 128 * n
                pieces = _pieces(n)
                halves = [h for h in range(2) if 1024 * (h + 1) > q0]
                ps = {}
                for h in halves:
                    ps[h] = ps_pool.tile(
                        [128, 1024], F32, tag="ps", name=f"ps{i}_{n}_{h}"
                    )
                for c0, w in pieces:
                    h = c0 // 1024
                    nc.tensor.matmul(
                        ps[h][:, c0 - 1024 * h : c0 - 1024 * h + w],
                        lhsT=kt[:, q0 : q0 + 128],
                        rhs=qt[:, c0 : c0 + w],
                        start=True,
                        stop=True,
                    )
                a_sb = a_pool.tile([128, L], BF16, tag="a", name=f"a{i}_{n}")
                for h in halves:
                    lo = max(q0, 1024 * h)
                    hi = 1024 * (h + 1)
                    nc.scalar.activation(
                        a_sb[:, lo:hi],
                        ps[h][:, lo - 1024 * h : hi - 1024 * h],
                        EXP,
                        bias=dls[:, n : n + 1],
                        scale=tts[:, 0:1],
                    )
                nc.vector.tensor_mul(
                    a_sb[:, q0 : q0 + 128], a_sb[:, q0 : q0 + 128], trimask[:]
                )
                for c0, w in pieces:
                    j = c0 // 512
                    nc.tensor.matmul(
                        oT[:, c0 : c0 + w],
                        lhsT=vt[:, n * 65 : n * 65 + 65],
                        rhs=a_sb[:, c0 : c0 + w],
                        start=(n == 0),
                        stop=(n == 4 * j + 3),
                    )
            # Head finalization: copy O^T' to SBUF, PE-transpose per 128-q
            # chunk so the denominator (row 64) becomes a per-partition
            # scalar, then reciprocal + scaled copy + store in [q, d] layout.
            nc.vector.tensor_copy(o_sb[:], oT[:, :])
            psT = po_pool.tile([128, L], F32, tag="oT", name=f"psT{i}")
            for t in range(NCH):
                nc.tensor.transpose(
                    psT[:, 128 * t : 128 * t + 65],
                    o_sb[:, 128 * t : 128 * t + 128],
                    ident65[:],
                )
            psT3 = psT.rearrange("p (t c) -> p t c", c=128)
            recipv = r_pool.tile([128, NCH], F32, tag="rr", name=f"rr{i}")
            nc.vector.reciprocal(recipv[:], psT3[:, :, 64])
            o2 = o_pool.tile([128, NCH, 64], F32, tag="o2", name=f"o2_{i}")
            nc.vector.scalar_tensor_tensor(
                out=o2[:],
                in0=psT3[:, :, 0:64],
                scalar=1.0,
                in1=recipv[:, :, None].to_broadcast([128, NCH, 64]),
                op0=MULT,
                op1=MULT,
            )
            nc.sync.dma_start(
                out[i].rearrange("(t p) d -> p t d", p=128), o2[:]
            )


_CACHED = None


def _build():
    global _CACHED
    if _CACHED is not None:
        return _CACHED
    nc = bacc.Bacc("TRN2", target_bir_lowering=False, debug=False)
    qT = nc.dram_tensor("qT", [HPC, 64, L], BF16, kind="ExternalInput").ap()
    kT = nc.dram_tensor("kT", [HPC, 64, L], BF16, kind="ExternalInput").ap()
    v1 = nc.dram_tensor("v1", [HPC, 128, NCH * 65], BF16, kind="ExternalInput").ap()
    dlt = nc.dram_tensor("dlt", [HPC, 128, NCH], F32, kind="ExternalInput").ap()
    tau_in = nc.dram_tensor("tau_in", [HPC, 1], F32, kind="ExternalInput").ap()
    out = nc.dram_tensor("out", [HPC, L, E], F32, kind="ExternalOutput").ap()
    with tile.TileContext(nc) as tc:
        _body(tc, qT, kT, v1, dlt, tau_in, out)
    nc.compile()
    _CACHED = nc
    return nc


def _prep_in_maps(queries, keys, values, tau, delta):
    """Shard + relayout the full inputs into 8 per-core input dicts."""
    queries = np.asarray(queries, dtype=np.float32)
    keys = np.asarray(keys, dtype=np.float32)
    values = np.asarray(values, dtype=np.float32)
    tau = np.asarray(tau, dtype=np.float32)
    delta = np.asarray(delta, dtype=np.float32)

    in_maps = []
    for core in range(NCORES):
        qTs = np.empty((HPC, 64, L), ml_dtypes.bfloat16)
        kTs = np.empty((HPC, 64, L), ml_dtypes.bfloat16)
        v1s = np.empty((HPC, 128, NCH * 65), ml_dtypes.bfloat16)
        dls = np.empty((HPC, 128, NCH), np.float32)
        tas = np.empty((HPC, 1), np.float32)
        for slot in range(HPC):
            g = core * HPC + slot
            b, h = divmod(g, H)
            qTs[slot] = queries[b, :, h, :].T
            kTs[slot] = keys[b, :, h, :].T
            v = values[b, :, h, :].reshape(NCH, 128, E).transpose(1, 0, 2)
            vv = np.concatenate([v, np.ones((128, NCH, 1), np.float32)], axis=2)
            v1s[slot] = vv.reshape(128, NCH * 65).astype(ml_dtypes.bfloat16)
            dls[slot] = delta[b].reshape(NCH, 128).T
            tas[slot, 0] = tau[b, 0]
        in_maps.append(
            {"qT": qTs, "kT": kTs, "v1": v1s, "dlt": dls, "tau_in": tas}
        )
    return in_maps


def _assemble(results):
    O = np.empty((B, L, H, E), np.float32)
    for core in range(NCORES):
        o = results[core]["out"]  # [HPC, L, E]
        for slot in range(HPC):
            g = core * HPC + slot
            b, h = divmod(g, H)
            O[b, :, h, :] = o[slot]
    return O


def run(inputs, trace=False, **kwargs):
    from concourse import bass_utils

    nc = _build()
    in_maps = _prep_in_maps(**inputs)
    res = bass_utils.run_bass_kernel_spmd(
        nc, in_maps, core_ids=list(range(NCORES)), trace=trace, **kwargs
    )
    return _assemble(res.results), res


def kernel(**inputs):
    return run(inputs, trace=False)[0]
